# revision 1
# baseline (speedup 1.0000x reference)
"""GAT layer (nn_GATLayer_32719060861314) as a Bass/Tile SPMD kernel on 8 trn2 cores.

Strategy (edge-sharded, dst-partitioned, gather-free), v2:
  - Node dsts range-partitioned over 8 cores (6250 dsts/core); each core owns
    all edges into its dst range (~200K), sorted by dst, grouped into 128-dst
    chunks, padded to 128-edge tiles.
  - Host stages per core (layout/dtype only): x[src]^T as a bf16 stream plus a
    16x-scaled fp8_e4m3 low-order correction stream, and BOTH one-hot
    selector streams in fp8 (exact 0/1): sds[w,e] (dst slot, for the q
    gather) and sel[e,w] (for the per-chunk segmented-sum aggregation).
  - Device per 128-edge tile:
      kv   = xs^T @ [Wk^T|Wv^T]_f16     (PE, bf16 x fp16, 256-col matmul)
      k   += xlo8^T @ (Wk^T/16)_f16     (PE, fp8 x fp16, 128-col; restores
                                         ~12-bit x mantissa on the k path)
      q_e  = sds^T @ q_chunk_f16        (PE, fp8 x fp16, 128 cols)
      q_sb = copy(q_e)  f16             (ACT, PSUM->SBUF, lossless)
      v_sb = copy(kv[v]) f16            (ACT, PSUM->SBUF)
      prod = k (PSUM) * q_sb            (DVE, fp32 out)
      scores = reduce_add(prod, 32)     (DVE)
    then per 32-tile block (batched):
      p    = exp(scores)                (ACT, one call per block, bf16)
      pv   = v_sb * p_bcast             (GPSIMD, bf16)
      agg += sel^T @ [pv | p]           (PE, fp8 x bf16, 132 cols,
                                         PSUM-accumulated per 128-dst chunk)
    finalize per chunk: out = pv_sum / p_sum, DMA out.
  - Output: concat of per-core [6250,128] blocks -> [50000,1,128].
"""
import math
import numpy as np
import ml_dtypes

import concourse.bass as bass
import concourse.tile as tile
from concourse import bacc, mybir
from concourse.bass_utils import run_bass_kernel_spmd

f32 = mybir.dt.float32
bf16 = mybir.dt.bfloat16
fp16 = mybir.dt.float16
fp8 = mybir.dt.float8e4

# problem shape (hardcoded per contract)
N = 50000
E = 1600000
D = 128
H = 4
NC = 8
NDST = N // NC          # 6250 dsts per core
CH = 128                # dsts per chunk
NCHUNK = (NDST + CH - 1) // CH   # 49
GT = 4                  # tiles per PSUM group
TB = 128                # edges per tile
BLK = 32                # tiles per stream block (1MB xs loads)
XLO_SCALE = 16.0        # xlo stream is (x - bf16(x)) * XLO_SCALE in fp8


def _schedule(src, dst):
    """Sort/pad edges per core; return common tile counts + per-core slot arrays."""
    core = dst // NDST
    dstl = dst % NDST
    per_core = []
    counts = np.zeros((NC, NCHUNK), np.int64)
    for c in range(NC):
        sel = np.nonzero(core == c)[0]
        order = np.argsort(dstl[sel], kind="stable")
        e = sel[order]
        chunk = dstl[e] // CH
        counts[c] = np.bincount(chunk, minlength=NCHUNK)
        per_core.append((e, dstl[e]))
    T = np.maximum(1, np.ceil(counts.max(axis=0) / TB).astype(np.int64))  # [NCHUNK]
    NT = int(T.sum())
    ETOT = NT * TB
    tile_base = np.concatenate([[0], np.cumsum(T)])  # chunk -> first tile
    slots_src = np.zeros((NC, ETOT), np.int64)
    dcmp = np.full((NC, ETOT), -1.0, np.float32)
    for c in range(NC):
        e, dl = per_core[c]
        pos = 0
        for j in range(NCHUNK):
            n = counts[c, j]
            base = int(tile_base[j]) * TB
            slots_src[c, base:base + n] = src[e[pos:pos + n]]
            dcmp[c, base:base + n] = (dl[pos:pos + n] - j * CH).astype(np.float32)
            pos += n
    return T, slots_src, dcmp


def _build(T, has_bias):
    """Emit the SPMD Bass program for common schedule T (tiles per chunk)."""
    T = [int(t) for t in T]
    NT = sum(T)
    ETOT = NT * TB
    NDSTP = NCHUNK * CH
    nc = bacc.Bacc("TRN2", target_bir_lowering=False, debug=False, num_devices=NC)
    xs_d = nc.dram_tensor("xs", [128, ETOT], bf16, kind="ExternalInput").ap()
    xlo_d = nc.dram_tensor("xlo", [128, ETOT], fp8, kind="ExternalInput").ap()
    sds_d = nc.dram_tensor("seldst", [128, ETOT], fp8, kind="ExternalInput").ap()
    sele_d = nc.dram_tensor("seledge", [128, ETOT], fp8, kind="ExternalInput").ap()
    xqT_d = nc.dram_tensor("xqT", [128, NDSTP], f32, kind="ExternalInput").ap()
    wkvT_d = nc.dram_tensor("wkvT", [128, 256], f32, kind="ExternalInput").ap()
    wqT_d = nc.dram_tensor("wqT", [128, 128], f32, kind="ExternalInput").ap()
    if has_bias:
        bkv_d = nc.dram_tensor("bkv", [1, 256], f32, kind="ExternalInput").ap()
        bq_d = nc.dram_tensor("bq", [1, 128], f32, kind="ExternalInput").ap()
        ones_d = nc.dram_tensor("ones", [1, 128], f32, kind="ExternalInput").ap()
    out_d = nc.dram_tensor("out", [NDST, 128], f32, kind="ExternalOutput").ap()

    with tile.TileContext(nc) as tc:
        with (
            tc.tile_pool(name="const", bufs=1) as cpool,
            tc.tile_pool(name="xs", bufs=3) as xspool,
            tc.tile_pool(name="xlo", bufs=3) as xlopool,
            tc.tile_pool(name="sds", bufs=3) as sdspool,
            tc.tile_pool(name="sele", bufs=3) as selepool,
            tc.tile_pool(name="qsb", bufs=3) as qsbpool,
            tc.tile_pool(name="vsb", bufs=2) as vsbpool,
            tc.tile_pool(name="prod", bufs=3) as prodpool,
            tc.tile_pool(name="sco", bufs=2) as scopool,
            tc.tile_pool(name="pvp", bufs=2) as pvppool,
            tc.tile_pool(name="qc", bufs=2) as qcpool,
            tc.tile_pool(name="fin", bufs=2) as finpool,
            tc.tile_pool(name="kvps", bufs=2, space="PSUM") as kvpspool,
            tc.tile_pool(name="qps", bufs=2, space="PSUM") as qpspool,
            tc.tile_pool(name="aggps", bufs=1, space="PSUM") as aggpool,
            tc.tile_pool(name="qcps", bufs=1, space="PSUM") as qcpspool,
        ):
            # ---- setup: load consts, cast weights ----
            wkvT = cpool.tile([128, 256], f32, tag="wkvT")
            wqT = cpool.tile([128, 128], f32, tag="wqT")
            nc.scalar.dma_start(out=wkvT[:], in_=wkvT_d[:])
            nc.scalar.dma_start(out=wqT[:], in_=wqT_d[:])
            wkv16 = cpool.tile([128, 256], fp16, tag="wkv16")
            wklo16 = cpool.tile([128, 128], fp16, tag="wklo16")
            nc.scalar.copy(out=wkv16[:], in_=wkvT[:])
            nc.scalar.mul(out=wklo16[:], in_=wkvT[:, 0:128], mul=1.0 / XLO_SCALE)
            if has_bias:
                bkv_sb = cpool.tile([1, 256], f32, tag="bkv")
                bq_sb = cpool.tile([1, 128], f32, tag="bq")
                ones_sb = cpool.tile([1, 128], f32, tag="ones")
                nc.scalar.dma_start(out=bkv_sb[:], in_=bkv_d[:])
                nc.scalar.dma_start(out=bq_sb[:], in_=bq_d[:])
                nc.scalar.dma_start(out=ones_sb[:], in_=ones_d[:])

            # ---- edge phase ----
            for j in range(NCHUNK):
                tbase = sum(T[:j])
                tj = T[j]
                agg = aggpool.tile([128, 132, 1], f32, tag="agg")

                # per-chunk q: q_chunk = xq_chunk @ Wq^T (fp32) -> fp16
                qc_ps = qcpspool.tile([128, 128, 1], f32, tag="qcps")
                xq_t = qcpool.tile([128, 128], f32, tag="xqt")
                nc.sync.dma_start(out=xq_t[:],
                                  in_=xqT_d[:, j * CH:(j + 1) * CH])
                nc.tensor.matmul(out=qc_ps[:, :, 0], lhsT=xq_t[:], rhs=wqT[:],
                                 start=True, stop=not has_bias,
                                 skip_group_check=True)
                if has_bias:
                    nc.tensor.matmul(out=qc_ps[:, :, 0], lhsT=ones_sb[0:1, :],
                                     rhs=bq_sb[0:1, :], start=False, stop=True,
                                     skip_group_check=True)
                qhi = qcpool.tile([128, 128], fp16, tag="qhi")
                nc.scalar.copy(out=qhi[:], in_=qc_ps[:, :, 0])

                t0b = 0
                first_tile = True
                while t0b < tj:
                    L = min(BLK, tj - t0b)
                    xs_b = xspool.tile([128, BLK * TB], bf16, tag="xs")
                    xlo_b = xlopool.tile([128, BLK * TB], fp8, tag="xlo")
                    sds_b = sdspool.tile([128, BLK * TB], fp8, tag="sds")
                    sel_b = selepool.tile([128, BLK * TB], fp8, tag="sele")
                    c0 = (tbase + t0b) * TB
                    nc.sync.dma_start(out=xs_b[:, : L * TB],
                                      in_=xs_d[:, c0:c0 + L * TB])
                    nc.sync.dma_start(out=xlo_b[:, : L * TB],
                                      in_=xlo_d[:, c0:c0 + L * TB])
                    nc.sync.dma_start(out=sds_b[:, : L * TB],
                                      in_=sds_d[:, c0:c0 + L * TB])
                    nc.sync.dma_start(out=sel_b[:, : L * TB],
                                      in_=sele_d[:, c0:c0 + L * TB])
                    # phase 1 over the block: projections + scores
                    v_b = vsbpool.tile([128, BLK, 128], fp16, tag="vsb")
                    sco_b = scopool.tile([128, BLK, H], f32, tag="sco")
                    done = 0
                    while done < L:
                        r = min(GT, L - done)
                        loc = done * TB
                        kv_ps = kvpspool.tile([128, GT, 256], f32, tag="kvps")
                        q_ps = qpspool.tile([128, GT, 128], f32, tag="qps")
                        for i in range(r):
                            sl = slice(loc + i * TB, loc + (i + 1) * TB)
                            nc.tensor.matmul(out=kv_ps[:, i, :],
                                             lhsT=xs_b[:, sl], rhs=wkv16[:],
                                             start=True, stop=False,
                                             skip_group_check=True)
                            nc.tensor.matmul(out=kv_ps[:, i, 0:128],
                                             lhsT=xlo_b[:, sl], rhs=wklo16[:],
                                             start=False, stop=not has_bias,
                                             skip_group_check=True)
                            if has_bias:
                                nc.tensor.matmul(out=kv_ps[:, i, :],
                                                 lhsT=ones_sb[0:1, :],
                                                 rhs=bkv_sb[0:1, :], start=False,
                                                 stop=True, skip_group_check=True)
                            nc.tensor.matmul(out=q_ps[:, i, :],
                                             lhsT=sds_b[:, sl], rhs=qhi[:],
                                             start=True, stop=True,
                                             skip_group_check=True)
                        q_sb = qsbpool.tile([128, GT, 128], fp16, tag="qsb")
                        nc.scalar.copy(out=q_sb[:, :r, :], in_=q_ps[:, :r, :])
                        nc.scalar.copy(out=v_b[:, done:done + r, :],
                                       in_=kv_ps[:, :r, 128:256])
                        prod = prodpool.tile([128, GT, H, 32], f32, tag="prod")
                        nc.vector.tensor_tensor(
                            out=prod[:, :r],
                            in0=kv_ps[:, :r, 0:128]
                                .rearrange("p r (h c) -> p r h c", h=H),
                            in1=q_sb[:, :r, :]
                                .rearrange("p r (h c) -> p r h c", h=H),
                            op=mybir.AluOpType.mult,
                        )
                        nc.vector.tensor_reduce(
                            out=sco_b[:, done:done + r, :], in_=prod[:, :r],
                            axis=mybir.AxisListType.X, op=mybir.AluOpType.add)
                        done += r

                    # phase 2 over the block: exp, pv, aggregate
                    pvp = pvppool.tile([128, BLK, 132], bf16, tag="pvp")
                    nc.scalar.activation(out=pvp[:, :L, 128:132],
                                         in_=sco_b[:, :L, :],
                                         func=mybir.ActivationFunctionType.Exp)
                    nc.gpsimd.tensor_tensor(
                        out=pvp[:, :L, 0:128].rearrange("p t (h c) -> p t h c",
                                                        h=H),
                        in0=v_b[:, :L, :].rearrange("p t (h c) -> p t h c", h=H),
                        in1=pvp[:, :L, 128:132].rearrange("p t (h o) -> p t h o",
                                                          o=1)
                            .to_broadcast([128, L, H, 32]),
                        op=mybir.AluOpType.mult,
                    )
                    for i in range(L):
                        nc.tensor.matmul(out=agg[:, :, 0],
                                         lhsT=sel_b[:, i * TB:(i + 1) * TB],
                                         rhs=pvp[:, i, :],
                                         start=first_tile,
                                         stop=(t0b + i == tj - 1),
                                         skip_group_check=True)
                        first_tile = False
                    t0b += L

                # finalize chunk
                den = finpool.tile([128, H, 1], f32, tag="den")
                nc.vector.tensor_scalar_max(den[:], agg[:, 128:132, :], 1e-30)
                rec = finpool.tile([128, H, 1], f32, tag="rec")
                nc.vector.reciprocal(rec[:], den[:])
                outn = finpool.tile([128, H, 32], f32, tag="outn")
                nc.vector.tensor_tensor(
                    out=outn[:],
                    in0=agg[:, 0:128, 0].rearrange("p (h c) -> p h c", h=H),
                    in1=rec[:].to_broadcast([128, H, 32]),
                    op=mybir.AluOpType.mult,
                )
                rows = min(CH, NDST - j * CH)
                nc.sync.dma_start(
                    out=out_d[j * CH: j * CH + rows, :],
                    in_=outn[:rows].rearrange("p h c -> p (h c)"),
                )
    nc.compile()
    return nc


def kernel(**inputs):
    x = np.ascontiguousarray(np.asarray(inputs["x"], np.float32))
    Wk = np.ascontiguousarray(np.asarray(inputs["Wk"], np.float32))
    Wq = np.ascontiguousarray(np.asarray(inputs["Wq"], np.float32))
    Wv = np.ascontiguousarray(np.asarray(inputs["Wv"], np.float32))
    bk = np.asarray(inputs["bk"], np.float32)
    bq = np.asarray(inputs["bq"], np.float32)
    bv = np.asarray(inputs["bv"], np.float32)
    src = np.asarray(inputs["src"]).astype(np.int64)
    dst = np.asarray(inputs["dst"]).astype(np.int64)

    has_bias = bool(bk.any() or bq.any() or bv.any())
    T, slots_src, dcmp = _schedule(src, dst)
    nc = _build(T, has_bias)

    NDSTP = NCHUNK * CH
    ETOT = slots_src.shape[1]
    NT = ETOT // TB
    wkvT = np.ascontiguousarray(np.concatenate([Wk.T, Wv.T], axis=1))
    wqT = np.ascontiguousarray(Wq.T)
    x_bf = x.astype(ml_dtypes.bfloat16)
    x_lo8 = ((x - x_bf.astype(np.float32)) * XLO_SCALE).astype(
        ml_dtypes.float8_e4m3fn)
    in_maps = []
    for c in range(NC):
        xsT = np.ascontiguousarray(x_bf[slots_src[c]].T)      # [128, ETOT] bf16
        xloT = np.ascontiguousarray(x_lo8[slots_src[c]].T)    # [128, ETOT] fp8
        dci = dcmp[c].astype(np.int64)
        cols = np.nonzero(dci >= 0)[0]
        sds = np.zeros((128, ETOT), ml_dtypes.float8_e4m3fn)
        sds[dci[cols], cols] = 1
        # sel[e, (t, w)]: one-hot over dst slot w for edge-slot e of tile t
        sel = np.zeros((NT, 128, 128), ml_dtypes.float8_e4m3fn)
        sel[cols // TB, cols % TB, dci[cols]] = 1
        sel = np.ascontiguousarray(sel.transpose(1, 0, 2).reshape(128, ETOT))
        xq = np.zeros((128, NDSTP), np.float32)
        xq[:, :NDST] = x[c * NDST:(c + 1) * NDST].T
        m = {
            "xs": xsT,
            "xlo": xloT,
            "seldst": sds,
            "seledge": sel,
            "xqT": xq,
            "wkvT": wkvT, "wqT": wqT,
        }
        if has_bias:
            m["bkv"] = np.concatenate([bk, bv]).reshape(1, 256).astype(np.float32)
            m["bq"] = bq.reshape(1, 128).astype(np.float32)
            m["ones"] = np.ones((1, 128), np.float32)
        in_maps.append(m)

    import os
    trace_dir = os.environ.get("BASS_GAT_TRACE")
    kw = {}
    if trace_dir:
        os.makedirs(trace_dir, exist_ok=True)
        kw = dict(trace=True, tmpdir=trace_dir)
    res = None
    for attempt in range(3):
        try:
            res = run_bass_kernel_spmd(nc, in_maps, core_ids=list(range(NC)), **kw)
            break
        except Exception:
            if attempt == 2:
                raise
            import time as _time
            _time.sleep(2.0)
    if trace_dir and res.exec_time_ns is not None:
        print(f"HW exec time: {res.exec_time_ns} ns")
    out = np.concatenate([res.results[c]["out"] for c in range(NC)], axis=0)
    return out.reshape(N, 1, D).astype(np.float32)


if __name__ == "__main__":
    rng = np.random.default_rng(0)
    ins = {
        "x": rng.standard_normal((N, D), np.float32),
        "Wk": (rng.standard_normal((D, D)) / math.sqrt(D)).astype(np.float32),
        "bk": np.zeros(D, np.float32),
        "Wq": (rng.standard_normal((D, D)) / math.sqrt(D)).astype(np.float32),
        "bq": np.zeros(D, np.float32),
        "Wv": (rng.standard_normal((D, D)) / math.sqrt(D)).astype(np.float32),
        "bv": np.zeros(D, np.float32),
        "src": rng.integers(0, N, E).astype(np.int32),
        "dst": rng.integers(0, N, E).astype(np.int32),
    }
    out = kernel(**ins)
    print("out", out.shape, out.dtype, np.abs(out).max())



# revision 2
# speedup vs baseline: 1.0222x; 1.0222x over previous
"""GAT layer (nn_GATLayer_32719060861314) as a Bass/Tile SPMD kernel on 8 trn2 cores.

Strategy (edge-sharded, dst-partitioned, gather-free), v3:
  - Node dsts range-partitioned over 8 cores (6250 dsts/core); each core owns
    all edges into its dst range (~200K), sorted by dst, grouped into 128-dst
    chunks, padded to 128-edge tiles.
  - Host stages per core (layout/dtype only): x[src]^T as an fp16 stream plus
    BOTH one-hot selector streams in fp8 (exact 0/1): sds[w,e] (dst slot, for
    the q gather) and sel[e,w] (for the per-chunk segmented-sum aggregation).
  - DMA: xs rides the Activation HWDGE queue; sds/sele/xq ride the SP queue
    (two hardware queues in parallel instead of one).
  - Device per 128-edge tile:
      kv   = xs^T @ [Wk^T|Wv^T]_f16     (PE, fp16 x fp16, 256-col matmul)
      q_e  = sds^T @ q_chunk_f16        (PE, fp8 x fp16, 128 cols)
      q_sb = copy(q_e)  f16             (ACT, PSUM->SBUF, lossless)
      v_sb = copy(kv[v]) f16            (ACT, PSUM->SBUF)
      prod = k (PSUM) * q_sb            (DVE, fp32 out)
      scores = reduce_add(prod, 32)     (DVE)
    then per 32-tile block (batched):
      p    = exp(scores)                (ACT, one call per block, bf16)
      pv   = v_sb * p_bcast             (GPSIMD, bf16)
      agg += sel^T @ [pv | p]           (PE, fp8 x bf16, 132 cols,
                                         PSUM-accumulated per 128-dst chunk)
    finalize per chunk: out = pv_sum / p_sum, DMA out.
  - Output: concat of per-core [6250,128] blocks -> [50000,1,128].
"""
import math
import numpy as np
import ml_dtypes

import concourse.bass as bass
import concourse.tile as tile
from concourse import bacc, mybir
from concourse.bass_utils import run_bass_kernel_spmd

f32 = mybir.dt.float32
bf16 = mybir.dt.bfloat16
fp16 = mybir.dt.float16
fp8 = mybir.dt.float8e4

# problem shape (hardcoded per contract)
N = 50000
E = 1600000
D = 128
H = 4
NC = 8
NDST = N // NC          # 6250 dsts per core
CH = 128                # dsts per chunk
NCHUNK = (NDST + CH - 1) // CH   # 49
GT = 4                  # tiles per PSUM group
TB = 128                # edges per tile
BLK = 32                # tiles per stream block (1MB xs loads)


def _schedule(src, dst):
    """Sort/pad edges per core; return common tile counts + per-core slot arrays."""
    core = dst // NDST
    dstl = dst % NDST
    per_core = []
    counts = np.zeros((NC, NCHUNK), np.int64)
    for c in range(NC):
        sel = np.nonzero(core == c)[0]
        order = np.argsort(dstl[sel], kind="stable")
        e = sel[order]
        chunk = dstl[e] // CH
        counts[c] = np.bincount(chunk, minlength=NCHUNK)
        per_core.append((e, dstl[e]))
    T = np.maximum(1, np.ceil(counts.max(axis=0) / TB).astype(np.int64))  # [NCHUNK]
    NT = int(T.sum())
    ETOT = NT * TB
    tile_base = np.concatenate([[0], np.cumsum(T)])  # chunk -> first tile
    slots_src = np.zeros((NC, ETOT), np.int64)
    dcmp = np.full((NC, ETOT), -1.0, np.float32)
    for c in range(NC):
        e, dl = per_core[c]
        pos = 0
        for j in range(NCHUNK):
            n = counts[c, j]
            base = int(tile_base[j]) * TB
            slots_src[c, base:base + n] = src[e[pos:pos + n]]
            dcmp[c, base:base + n] = (dl[pos:pos + n] - j * CH).astype(np.float32)
            pos += n
    return T, slots_src, dcmp


def _build(T, has_bias):
    """Emit the SPMD Bass program for common schedule T (tiles per chunk)."""
    T = [int(t) for t in T]
    NT = sum(T)
    ETOT = NT * TB
    NDSTP = NCHUNK * CH
    nc = bacc.Bacc("TRN2", target_bir_lowering=False, debug=False, num_devices=NC)
    xs_d = nc.dram_tensor("xs", [128, ETOT], fp16, kind="ExternalInput").ap()
    sds_d = nc.dram_tensor("seldst", [128, ETOT], fp8, kind="ExternalInput").ap()
    sele_d = nc.dram_tensor("seledge", [128, ETOT], fp8, kind="ExternalInput").ap()
    xqT_d = nc.dram_tensor("xqT", [128, NDSTP], f32, kind="ExternalInput").ap()
    wkvT_d = nc.dram_tensor("wkvT", [128, 256], f32, kind="ExternalInput").ap()
    wqT_d = nc.dram_tensor("wqT", [128, 128], f32, kind="ExternalInput").ap()
    if has_bias:
        bkv_d = nc.dram_tensor("bkv", [1, 256], f32, kind="ExternalInput").ap()
        bq_d = nc.dram_tensor("bq", [1, 128], f32, kind="ExternalInput").ap()
        ones_d = nc.dram_tensor("ones", [1, 128], f32, kind="ExternalInput").ap()
    out_d = nc.dram_tensor("out", [NDST, 128], f32, kind="ExternalOutput").ap()

    with tile.TileContext(nc) as tc:
        with (
            tc.tile_pool(name="const", bufs=1) as cpool,
            tc.tile_pool(name="xs", bufs=3) as xspool,
            tc.tile_pool(name="sds", bufs=3) as sdspool,
            tc.tile_pool(name="sele", bufs=3) as selepool,
            tc.tile_pool(name="qsb", bufs=3) as qsbpool,
            tc.tile_pool(name="vsb", bufs=2) as vsbpool,
            tc.tile_pool(name="prod", bufs=3) as prodpool,
            tc.tile_pool(name="sco", bufs=2) as scopool,
            tc.tile_pool(name="pvp", bufs=2) as pvppool,
            tc.tile_pool(name="qc", bufs=2) as qcpool,
            tc.tile_pool(name="fin", bufs=2) as finpool,
            tc.tile_pool(name="kvps", bufs=2, space="PSUM") as kvpspool,
            tc.tile_pool(name="qps", bufs=2, space="PSUM") as qpspool,
            tc.tile_pool(name="aggps", bufs=1, space="PSUM") as aggpool,
            tc.tile_pool(name="qcps", bufs=1, space="PSUM") as qcpspool,
        ):
            # ---- setup: load consts, cast weights ----
            wkvT = cpool.tile([128, 256], f32, tag="wkvT")
            wqT = cpool.tile([128, 128], f32, tag="wqT")
            nc.scalar.dma_start(out=wkvT[:], in_=wkvT_d[:])
            nc.scalar.dma_start(out=wqT[:], in_=wqT_d[:])
            wkv16 = cpool.tile([128, 256], fp16, tag="wkv16")
            nc.scalar.copy(out=wkv16[:], in_=wkvT[:])
            if has_bias:
                bkv_sb = cpool.tile([1, 256], f32, tag="bkv")
                bq_sb = cpool.tile([1, 128], f32, tag="bq")
                ones_sb = cpool.tile([1, 128], f32, tag="ones")
                nc.scalar.dma_start(out=bkv_sb[:], in_=bkv_d[:])
                nc.scalar.dma_start(out=bq_sb[:], in_=bq_d[:])
                nc.scalar.dma_start(out=ones_sb[:], in_=ones_d[:])

            # ---- edge phase ----
            for j in range(NCHUNK):
                tbase = sum(T[:j])
                tj = T[j]
                agg = aggpool.tile([128, 132, 1], f32, tag="agg")

                # per-chunk q: q_chunk = xq_chunk @ Wq^T (fp32) -> fp16
                qc_ps = qcpspool.tile([128, 128, 1], f32, tag="qcps")
                xq_t = qcpool.tile([128, 128], f32, tag="xqt")
                nc.sync.dma_start(out=xq_t[:],
                                  in_=xqT_d[:, j * CH:(j + 1) * CH])
                nc.tensor.matmul(out=qc_ps[:, :, 0], lhsT=xq_t[:], rhs=wqT[:],
                                 start=True, stop=not has_bias,
                                 skip_group_check=True)
                if has_bias:
                    nc.tensor.matmul(out=qc_ps[:, :, 0], lhsT=ones_sb[0:1, :],
                                     rhs=bq_sb[0:1, :], start=False, stop=True,
                                     skip_group_check=True)
                qhi = qcpool.tile([128, 128], fp16, tag="qhi")
                nc.scalar.copy(out=qhi[:], in_=qc_ps[:, :, 0])

                t0b = 0
                first_tile = True
                while t0b < tj:
                    L = min(BLK, tj - t0b)
                    xs_b = xspool.tile([128, BLK * TB], fp16, tag="xs")
                    sds_b = sdspool.tile([128, BLK * TB], fp8, tag="sds")
                    sel_b = selepool.tile([128, BLK * TB], fp8, tag="sele")
                    c0 = (tbase + t0b) * TB
                    nc.scalar.dma_start(out=xs_b[:, : L * TB],
                                        in_=xs_d[:, c0:c0 + L * TB])
                    nc.sync.dma_start(out=sds_b[:, : L * TB],
                                      in_=sds_d[:, c0:c0 + L * TB])
                    nc.sync.dma_start(out=sel_b[:, : L * TB],
                                      in_=sele_d[:, c0:c0 + L * TB])
                    # phase 1 over the block: projections + scores
                    v_b = vsbpool.tile([128, BLK, 128], fp16, tag="vsb")
                    sco_b = scopool.tile([128, BLK, H], f32, tag="sco")
                    done = 0
                    while done < L:
                        r = min(GT, L - done)
                        loc = done * TB
                        kv_ps = kvpspool.tile([128, GT, 256], f32, tag="kvps")
                        q_ps = qpspool.tile([128, GT, 128], f32, tag="qps")
                        for i in range(r):
                            sl = slice(loc + i * TB, loc + (i + 1) * TB)
                            nc.tensor.matmul(out=kv_ps[:, i, :],
                                             lhsT=xs_b[:, sl], rhs=wkv16[:],
                                             start=True, stop=not has_bias,
                                             skip_group_check=True)
                            if has_bias:
                                nc.tensor.matmul(out=kv_ps[:, i, :],
                                                 lhsT=ones_sb[0:1, :],
                                                 rhs=bkv_sb[0:1, :], start=False,
                                                 stop=True, skip_group_check=True)
                            nc.tensor.matmul(out=q_ps[:, i, :],
                                             lhsT=sds_b[:, sl], rhs=qhi[:],
                                             start=True, stop=True,
                                             skip_group_check=True)
                        q_sb = qsbpool.tile([128, GT, 128], fp16, tag="qsb")
                        nc.scalar.copy(out=q_sb[:, :r, :], in_=q_ps[:, :r, :])
                        nc.scalar.copy(out=v_b[:, done:done + r, :],
                                       in_=kv_ps[:, :r, 128:256])
                        prod = prodpool.tile([128, GT, H, 32], f32, tag="prod")
                        nc.vector.tensor_tensor(
                            out=prod[:, :r],
                            in0=kv_ps[:, :r, 0:128]
                                .rearrange("p r (h c) -> p r h c", h=H),
                            in1=q_sb[:, :r, :]
                                .rearrange("p r (h c) -> p r h c", h=H),
                            op=mybir.AluOpType.mult,
                        )
                        nc.vector.tensor_reduce(
                            out=sco_b[:, done:done + r, :], in_=prod[:, :r],
                            axis=mybir.AxisListType.X, op=mybir.AluOpType.add)
                        done += r

                    # phase 2 over the block: exp, pv, aggregate
                    pvp = pvppool.tile([128, BLK, 132], bf16, tag="pvp")
                    nc.scalar.activation(out=pvp[:, :L, 128:132],
                                         in_=sco_b[:, :L, :],
                                         func=mybir.ActivationFunctionType.Exp)
                    nc.gpsimd.tensor_tensor(
                        out=pvp[:, :L, 0:128].rearrange("p t (h c) -> p t h c",
                                                        h=H),
                        in0=v_b[:, :L, :].rearrange("p t (h c) -> p t h c", h=H),
                        in1=pvp[:, :L, 128:132].rearrange("p t (h o) -> p t h o",
                                                          o=1)
                            .to_broadcast([128, L, H, 32]),
                        op=mybir.AluOpType.mult,
                    )
                    for i in range(L):
                        nc.tensor.matmul(out=agg[:, :, 0],
                                         lhsT=sel_b[:, i * TB:(i + 1) * TB],
                                         rhs=pvp[:, i, :],
                                         start=first_tile,
                                         stop=(t0b + i == tj - 1),
                                         skip_group_check=True)
                        first_tile = False
                    t0b += L

                # finalize chunk
                den = finpool.tile([128, H, 1], f32, tag="den")
                nc.vector.tensor_scalar_max(den[:], agg[:, 128:132, :], 1e-30)
                rec = finpool.tile([128, H, 1], f32, tag="rec")
                nc.vector.reciprocal(rec[:], den[:])
                outn = finpool.tile([128, H, 32], f32, tag="outn")
                nc.vector.tensor_tensor(
                    out=outn[:],
                    in0=agg[:, 0:128, 0].rearrange("p (h c) -> p h c", h=H),
                    in1=rec[:].to_broadcast([128, H, 32]),
                    op=mybir.AluOpType.mult,
                )
                rows = min(CH, NDST - j * CH)
                nc.sync.dma_start(
                    out=out_d[j * CH: j * CH + rows, :],
                    in_=outn[:rows].rearrange("p h c -> p (h c)"),
                )
    nc.compile()
    return nc


def kernel(**inputs):
    x = np.ascontiguousarray(np.asarray(inputs["x"], np.float32))
    Wk = np.ascontiguousarray(np.asarray(inputs["Wk"], np.float32))
    Wq = np.ascontiguousarray(np.asarray(inputs["Wq"], np.float32))
    Wv = np.ascontiguousarray(np.asarray(inputs["Wv"], np.float32))
    bk = np.asarray(inputs["bk"], np.float32)
    bq = np.asarray(inputs["bq"], np.float32)
    bv = np.asarray(inputs["bv"], np.float32)
    src = np.asarray(inputs["src"]).astype(np.int64)
    dst = np.asarray(inputs["dst"]).astype(np.int64)

    has_bias = bool(bk.any() or bq.any() or bv.any())
    T, slots_src, dcmp = _schedule(src, dst)
    nc = _build(T, has_bias)

    NDSTP = NCHUNK * CH
    ETOT = slots_src.shape[1]
    NT = ETOT // TB
    wkvT = np.ascontiguousarray(np.concatenate([Wk.T, Wv.T], axis=1))
    wqT = np.ascontiguousarray(Wq.T)
    x_f16 = x.astype(np.float16)
    in_maps = []
    for c in range(NC):
        xsT = np.ascontiguousarray(x_f16[slots_src[c]].T)     # [128, ETOT] fp16
        dci = dcmp[c].astype(np.int64)
        cols = np.nonzero(dci >= 0)[0]
        sds = np.zeros((128, ETOT), ml_dtypes.float8_e4m3fn)
        sds[dci[cols], cols] = 1
        # sel[e, (t, w)]: one-hot over dst slot w for edge-slot e of tile t
        sel = np.zeros((NT, 128, 128), ml_dtypes.float8_e4m3fn)
        sel[cols // TB, cols % TB, dci[cols]] = 1
        sel = np.ascontiguousarray(sel.transpose(1, 0, 2).reshape(128, ETOT))
        xq = np.zeros((128, NDSTP), np.float32)
        xq[:, :NDST] = x[c * NDST:(c + 1) * NDST].T
        m = {
            "xs": xsT,
            "seldst": sds,
            "seledge": sel,
            "xqT": xq,
            "wkvT": wkvT, "wqT": wqT,
        }
        if has_bias:
            m["bkv"] = np.concatenate([bk, bv]).reshape(1, 256).astype(np.float32)
            m["bq"] = bq.reshape(1, 128).astype(np.float32)
            m["ones"] = np.ones((1, 128), np.float32)
        in_maps.append(m)

    import os
    trace_dir = os.environ.get("BASS_GAT_TRACE")
    kw = {}
    if trace_dir:
        os.makedirs(trace_dir, exist_ok=True)
        kw = dict(trace=True, tmpdir=trace_dir)
    res = None
    for attempt in range(3):
        try:
            res = run_bass_kernel_spmd(nc, in_maps, core_ids=list(range(NC)), **kw)
            break
        except Exception:
            if attempt == 2:
                raise
            import time as _time
            _time.sleep(2.0)
    if trace_dir and res.exec_time_ns is not None:
        print(f"HW exec time: {res.exec_time_ns} ns")
    out = np.concatenate([res.results[c]["out"] for c in range(NC)], axis=0)
    return out.reshape(N, 1, D).astype(np.float32)


if __name__ == "__main__":
    rng = np.random.default_rng(0)
    ins = {
        "x": rng.standard_normal((N, D), np.float32),
        "Wk": (rng.standard_normal((D, D)) / math.sqrt(D)).astype(np.float32),
        "bk": np.zeros(D, np.float32),
        "Wq": (rng.standard_normal((D, D)) / math.sqrt(D)).astype(np.float32),
        "bq": np.zeros(D, np.float32),
        "Wv": (rng.standard_normal((D, D)) / math.sqrt(D)).astype(np.float32),
        "bv": np.zeros(D, np.float32),
        "src": rng.integers(0, N, E).astype(np.int32),
        "dst": rng.integers(0, N, E).astype(np.int32),
    }
    out = kernel(**ins)
    print("out", out.shape, out.dtype, np.abs(out).max())
